# revision 1
# baseline (speedup 1.0000x reference)
"""Approximate depthwise WAConv2d on 8 Trainium2 NeuronCores.

Math: the reference computes, per 7x7 depthwise patch,
    y = sum_k factor(a,b) * x*w,  factor = (a+b-1)/(a*b),
    a = mantissa_map(x), b = mantissa_map(w).
Since factor*x*w = x*(w/b) + (x/a)*w - (x/a)*(w/b), the whole op is exactly
    y = DWConv(x, w') + DWConv(x', w - w'),   x' = x/a, w' = w/b
i.e. two exact depthwise convolutions, which we fuse into a single
TensorEngine matmul accumulation by stacking along the contraction dim.

Device formulation (per channel c, kernel-column j):
    out[y, (b,x)] += sum_u T_j[u, y] * Xp_b[u, x+j]
where u runs over 62 zero-padded input rows for x (partitions 0..61) and 62
for x' (partitions 62..123), and T_j is the banded Toeplitz expansion
T_j[y+i, y] = w'[c,i,j], T_j[62+y+i, y] = (w-w')[c,i,j]. 7 j-shifted matmuls
(K=124, M=56, N=4*56=224) accumulate in PSUM. Channels are sharded 24/core.
"""

import os
import sys

import numpy as np

if "/opt/trn_rl_repo" not in sys.path:
    sys.path.insert(0, "/opt/trn_rl_repo")

import ml_dtypes

B, C, H, W = 4, 192, 56, 56
KH = KW = 7
PAD = 3
HP = H + 2 * PAD  # 62 padded rows/cols
N_CORES = 8
CP = C // N_CORES  # 24 channels per core
EPS = 1e-7

BF16 = ml_dtypes.bfloat16

_CACHE = {}
LAST_RESULT = None


def _mantissa_map(v):
    a = np.abs(v, dtype=np.float32) + np.float32(EPS)
    base = a * np.exp2(-np.floor(np.log2(a))) - np.float32(1.0)
    return np.where(base >= 0.5, (1.0 + base) * 0.5, 1.0 + base).astype(np.float32)


def _build_graph():
    """Build + compile the Bass graph once per process."""
    if "nc" in _CACHE:
        return _CACHE["nc"]

    import concourse.tile as tile
    from concourse import bacc, mybir

    nc = bacc.Bacc(
        "TRN2",
        target_bir_lowering=False,
        debug=False,
        enable_asserts=False,
        num_devices=N_CORES,
    )
    xin = nc.dram_tensor(
        "xin", [CP, 2 * HP, B, HP], mybir.dt.bfloat16, kind="ExternalInput"
    ).ap()
    tw = nc.dram_tensor(
        "tw", [CP, 2 * HP, KW, H], mybir.dt.bfloat16, kind="ExternalInput"
    ).ap()
    out = nc.dram_tensor(
        "out", [CP, H, B, W], mybir.dt.float32, kind="ExternalOutput"
    ).ap()

    with tile.TileContext(nc) as tc:
        with (
            tc.tile_pool(name="xp", bufs=CP) as xpool,
            tc.tile_pool(name="tp", bufs=CP) as tpool,
            tc.tile_pool(name="pp", bufs=8, space="PSUM") as ppool,
            tc.tile_pool(name="op", bufs=CP) as opool,
        ):
            for c in range(CP):
                xt = xpool.tile([2 * HP, B, HP], mybir.dt.bfloat16)
                nc.sync.dma_start(out=xt[:], in_=xin[c])
                tt = tpool.tile([2 * HP, KW, H], mybir.dt.bfloat16)
                nc.sync.dma_start(out=tt[:], in_=tw[c])
                ps = ppool.tile([H, B, W], mybir.dt.float32)
                for j in range(KW):
                    nc.tensor.matmul(
                        ps[:],
                        tt[:, j, :],
                        xt[:, :, j : j + W],
                        start=(j == 0),
                        stop=(j == KW - 1),
                    )
                ot = opool.tile([H, B, W], mybir.dt.float32)
                nc.vector.tensor_copy(ot[:], ps[:])
                nc.sync.dma_start(out=out[c], in_=ot[:])

    nc.compile()
    _CACHE["nc"] = nc
    return nc


def kernel(x, weight):
    global LAST_RESULT
    from concourse.bass_utils import run_bass_kernel_spmd

    x = np.asarray(x, dtype=np.float32)
    w = np.asarray(weight, dtype=np.float32)
    assert x.shape == (B, C, H, W) and w.shape == (C, 1, KH, KW)

    # --- host-side exact prep (mirrors reference arithmetic in f32) ---
    x2 = (x / _mantissa_map(x)).astype(np.float32)  # x' : signed powers of two
    wr = w.reshape(C, KH, KW)
    wq = (wr / _mantissa_map(wr)).astype(np.float32)  # w'
    w2 = (wr - wq).astype(np.float32)  # w - w'

    # padded, K-stacked input layout: xin_all[c, u, b, col]
    xp = np.zeros((B, C, HP, HP), np.float32)
    xp[:, :, PAD : PAD + H, PAD : PAD + W] = x
    x2p = np.zeros((B, C, HP, HP), np.float32)
    x2p[:, :, PAD : PAD + H, PAD : PAD + W] = x2
    xin_all = np.empty((C, 2 * HP, B, HP), dtype=BF16)
    xin_all[:, :HP] = xp.transpose(1, 2, 0, 3).astype(BF16)
    xin_all[:, HP:] = x2p.transpose(1, 2, 0, 3).astype(BF16)

    # banded Toeplitz expansion: tw_all[c, u, j, y]
    tw_all = np.zeros((C, 2 * HP, KW, H), np.float32)
    yidx = np.arange(H)
    for i in range(KH):
        # advanced indexing on dims 1,3 -> result dims (y, c, j)
        tw_all[:, yidx + i, :, yidx] = np.broadcast_to(wq[:, i, :], (H, C, KW))
        tw_all[:, HP + yidx + i, :, yidx] = np.broadcast_to(w2[:, i, :], (H, C, KW))
    tw_all = tw_all.astype(BF16)

    nc = _build_graph()
    in_maps = [
        {
            "xin": np.ascontiguousarray(xin_all[k * CP : (k + 1) * CP]),
            "tw": np.ascontiguousarray(tw_all[k * CP : (k + 1) * CP]),
        }
        for k in range(N_CORES)
    ]
    res = run_bass_kernel_spmd(
        nc,
        in_maps,
        core_ids=list(range(N_CORES)),
        trace=bool(int(os.environ.get("KERNEL_TRACE", "0") or "0")),
    )
    LAST_RESULT = res

    y = np.stack([res.results[k]["out"] for k in range(N_CORES)])  # (8,CP,H,B,W)
    y = y.transpose(3, 0, 1, 2, 4).reshape(B, C, H, W)
    return np.ascontiguousarray(y.astype(np.float32))


# revision 4
# speedup vs baseline: 1.1796x; 1.1796x over previous
"""Approximate depthwise WAConv2d on 8 Trainium2 NeuronCores.

Math: the reference computes, per 7x7 depthwise patch,
    y = sum_k factor(a,b) * x*w,  factor = (a+b-1)/(a*b),
    a = mantissa_map(x), b = mantissa_map(w).
Since factor*x*w = x*(w/b) + (x/a)*w - (x/a)*(w/b), the whole op is exactly
    y = DWConv(x, w') + DWConv(x', w - w'),   x' = x/a, w' = w/b
i.e. two exact depthwise convolutions, which we fuse into a single
TensorEngine matmul accumulation by stacking along the contraction dim.

Device formulation (per channel c, kernel-column j):
    out[y, (b,x)] += sum_u T_j[u, y] * Xp_b[u, x+j]
where u runs over 62 zero-padded input rows for x (partitions 0..61) and 62
for x' (partitions 62..123), and T_j is the banded Toeplitz expansion
T_j[y+i, y] = w'[c,i,j], T_j[62+y+i, y] = (w-w')[c,i,j]. 7 j-shifted matmuls
(K=124, M=56, N=4*56=224) accumulate in PSUM. Channels are sharded 24/core.
"""

import os
import sys

import numpy as np

if "/opt/trn_rl_repo" not in sys.path:
    sys.path.insert(0, "/opt/trn_rl_repo")

import ml_dtypes

B, C, H, W = 4, 192, 56, 56
KH = KW = 7
PAD = 3
HP = H + 2 * PAD  # 62 padded rows/cols
N_CORES = 8
CP = C // N_CORES  # 24 channels per core
EPS = 1e-7

BF16 = ml_dtypes.bfloat16

_CACHE = {}
LAST_RESULT = None


def _mantissa_map(v):
    a = np.abs(v, dtype=np.float32) + np.float32(EPS)
    base = a * np.exp2(-np.floor(np.log2(a))) - np.float32(1.0)
    return np.where(base >= 0.5, (1.0 + base) * 0.5, 1.0 + base).astype(np.float32)


CHUNK = 6  # channels per DMA chunk
N_CHUNKS = CP // CHUNK


def _build_graph():
    """Build + compile the Bass graph once per process."""
    if "nc" in _CACHE:
        return _CACHE["nc"]

    import concourse.tile as tile
    from concourse import bacc, mybir

    nc = bacc.Bacc(
        "TRN2",
        target_bir_lowering=False,
        debug=False,
        enable_asserts=False,
        num_devices=N_CORES,
    )
    # channel-minor layouts so chunk DMAs have multi-KB contiguous lines
    xin = nc.dram_tensor(
        "xin", [2 * HP, CP, B, HP], mybir.dt.bfloat16, kind="ExternalInput"
    ).ap()
    tw = nc.dram_tensor(
        "tw", [2 * HP, CP, KW, H], mybir.dt.bfloat16, kind="ExternalInput"
    ).ap()
    out = nc.dram_tensor(
        "out", [H, CP, B, W], mybir.dt.float32, kind="ExternalOutput"
    ).ap()

    with tile.TileContext(nc) as tc:
        with (
            tc.tile_pool(name="xp", bufs=N_CHUNKS) as xpool,
            tc.tile_pool(name="tp", bufs=N_CHUNKS) as tpool,
            tc.tile_pool(name="pp", bufs=8, space="PSUM") as ppool,
            tc.tile_pool(name="op", bufs=N_CHUNKS) as opool,
        ):
            for g in range(N_CHUNKS):
                cs = slice(g * CHUNK, (g + 1) * CHUNK)
                xt = xpool.tile([2 * HP, CHUNK, B, HP], mybir.dt.bfloat16)
                nc.sync.dma_start(out=xt[:], in_=xin[:, cs])
                tt = tpool.tile([2 * HP, CHUNK, KW, H], mybir.dt.bfloat16)
                nc.sync.dma_start(out=tt[:], in_=tw[:, cs])
                ot = opool.tile([H, CHUNK, B, W], mybir.dt.float32)
                for ci in range(CHUNK):
                    ps = ppool.tile([H, B, W], mybir.dt.float32)
                    for j in range(KW):
                        nc.tensor.matmul(
                            ps[:],
                            tt[:, ci, j, :],
                            xt[:, ci, :, j : j + W],
                            start=(j == 0),
                            stop=(j == KW - 1),
                        )
                    nc.vector.tensor_copy(ot[:, ci], ps[:])
                nc.sync.dma_start(out=out[:, cs], in_=ot[:])

    nc.compile()
    _CACHE["nc"] = nc
    return nc


def kernel(x, weight):
    global LAST_RESULT
    from concourse.bass_utils import run_bass_kernel_spmd

    x = np.asarray(x, dtype=np.float32)
    w = np.asarray(weight, dtype=np.float32)
    assert x.shape == (B, C, H, W) and w.shape == (C, 1, KH, KW)

    # --- host-side exact prep (mirrors reference arithmetic in f32) ---
    x2 = (x / _mantissa_map(x)).astype(np.float32)  # x' : signed powers of two
    wr = w.reshape(C, KH, KW)
    wq = (wr / _mantissa_map(wr)).astype(np.float32)  # w'
    w2 = (wr - wq).astype(np.float32)  # w - w'

    # padded, K-stacked input layout: xin_all[u, c, b, col]
    xp = np.zeros((B, C, HP, HP), np.float32)
    xp[:, :, PAD : PAD + H, PAD : PAD + W] = x
    x2p = np.zeros((B, C, HP, HP), np.float32)
    x2p[:, :, PAD : PAD + H, PAD : PAD + W] = x2
    xin_all = np.empty((2 * HP, C, B, HP), dtype=BF16)
    xin_all[:HP] = xp.transpose(2, 1, 0, 3).astype(BF16)
    xin_all[HP:] = x2p.transpose(2, 1, 0, 3).astype(BF16)

    # banded Toeplitz expansion: tw_all[u, c, j, y]
    tw_all = np.zeros((2 * HP, C, KW, H), np.float32)
    yidx = np.arange(H)
    for i in range(KH):
        # advanced indexing on dims 0,3 -> result dims (y, c, j)
        tw_all[yidx + i, :, :, yidx] = np.broadcast_to(wq[:, i, :], (H, C, KW))
        tw_all[HP + yidx + i, :, :, yidx] = np.broadcast_to(w2[:, i, :], (H, C, KW))
    tw_all = tw_all.astype(BF16)

    nc = _build_graph()
    in_maps = [
        {
            "xin": np.ascontiguousarray(xin_all[:, k * CP : (k + 1) * CP]),
            "tw": np.ascontiguousarray(tw_all[:, k * CP : (k + 1) * CP]),
        }
        for k in range(N_CORES)
    ]
    res = run_bass_kernel_spmd(
        nc,
        in_maps,
        core_ids=list(range(N_CORES)),
        trace=bool(int(os.environ.get("KERNEL_TRACE", "0") or "0")),
    )
    LAST_RESULT = res

    y = np.stack([res.results[k]["out"] for k in range(N_CORES)])  # (8,H,CP,B,W)
    y = y.transpose(3, 0, 2, 1, 4).reshape(B, C, H, W)
    return np.ascontiguousarray(y.astype(np.float32))


# revision 5
# speedup vs baseline: 1.4444x; 1.2245x over previous
"""Approximate depthwise WAConv2d on 8 Trainium2 NeuronCores.

Math: the reference computes, per 7x7 depthwise patch,
    y = sum_k factor(a,b) * x*w,  factor = (a+b-1)/(a*b),
    a = mantissa_map(x), b = mantissa_map(w).
Since factor*x*w = x*(w/b) + (x/a)*w - (x/a)*(w/b), the whole op is exactly
    y = DWConv(x, w') + DWConv(x', w - w'),   x' = x/a, w' = w/b
i.e. two exact depthwise convolutions, which we fuse into a single
TensorEngine matmul accumulation by stacking along the contraction dim.

Device formulation (per channel c, kernel-column j):
    out[y, (b,x)] += sum_u T_j[u, y] * Xp_b[u, x+j]
where u runs over 62 zero-padded input rows for x (partitions 0..61) and 62
for x' (partitions 62..123), and T_j is the banded Toeplitz expansion
T_j[y+i, y] = w'[c,i,j], T_j[62+y+i, y] = (w-w')[c,i,j]. 7 j-shifted matmuls
(K=124, M=56, N=4*56=224) accumulate in PSUM. Channels are sharded 24/core.
"""

import os
import sys

import numpy as np

if "/opt/trn_rl_repo" not in sys.path:
    sys.path.insert(0, "/opt/trn_rl_repo")

import ml_dtypes

B, C, H, W = 4, 192, 56, 56
KH = KW = 7
PAD = 3
HP = H + 2 * PAD  # 62 padded rows/cols
N_CORES = 8
CP = C // N_CORES  # 24 channels per core
EPS = 1e-7

BF16 = ml_dtypes.bfloat16

_CACHE = {}
LAST_RESULT = None


def _mantissa_map(v):
    a = np.abs(v, dtype=np.float32) + np.float32(EPS)
    base = a * np.exp2(-np.floor(np.log2(a))) - np.float32(1.0)
    return np.where(base >= 0.5, (1.0 + base) * 0.5, 1.0 + base).astype(np.float32)


CHUNK = 6  # channels per DMA chunk
N_CHUNKS = CP // CHUNK


def _build_graph():
    """Build + compile the Bass graph once per process."""
    if "nc" in _CACHE:
        return _CACHE["nc"]

    import concourse.tile as tile
    from concourse import bacc, mybir

    nc = bacc.Bacc(
        "TRN2",
        target_bir_lowering=False,
        debug=False,
        enable_asserts=False,
        num_devices=N_CORES,
    )
    # channel-minor layouts so chunk DMAs have multi-KB contiguous lines
    xin = nc.dram_tensor(
        "xin", [2 * HP, CP, B, HP], mybir.dt.bfloat16, kind="ExternalInput"
    ).ap()
    tw = nc.dram_tensor(
        "tw", [2 * HP, CP, KW, H], mybir.dt.bfloat16, kind="ExternalInput"
    ).ap()
    out = nc.dram_tensor(
        "out", [H, CP, B, W], mybir.dt.float32, kind="ExternalOutput"
    ).ap()

    with tile.TileContext(nc) as tc:
        with (
            tc.tile_pool(name="xp", bufs=N_CHUNKS) as xpool,
            tc.tile_pool(name="tp", bufs=N_CHUNKS) as tpool,
            tc.tile_pool(name="pp", bufs=8, space="PSUM") as ppool,
            tc.tile_pool(name="op", bufs=N_CHUNKS) as opool,
        ):
            for g in range(N_CHUNKS):
                cs = slice(g * CHUNK, (g + 1) * CHUNK)
                xt = xpool.tile([2 * HP, CHUNK, B, HP], mybir.dt.bfloat16)
                nc.gpsimd.dma_start(out=xt[:], in_=xin[:, cs])
                tt = tpool.tile([2 * HP, CHUNK, KW, H], mybir.dt.bfloat16)
                nc.gpsimd.dma_start(out=tt[:], in_=tw[:, cs])
                ot = opool.tile([H, CHUNK, B, W], mybir.dt.float32)
                for ci in range(CHUNK):
                    ps = ppool.tile([H, B, W], mybir.dt.float32)
                    for j in range(KW):
                        nc.tensor.matmul(
                            ps[:],
                            tt[:, ci, j, :],
                            xt[:, ci, :, j : j + W],
                            start=(j == 0),
                            stop=(j == KW - 1),
                        )
                    nc.vector.tensor_copy(ot[:, ci], ps[:])
                nc.sync.dma_start(out=out[:, cs], in_=ot[:])

    nc.compile()
    _CACHE["nc"] = nc
    return nc


def kernel(x, weight):
    global LAST_RESULT
    from concourse.bass_utils import run_bass_kernel_spmd

    x = np.asarray(x, dtype=np.float32)
    w = np.asarray(weight, dtype=np.float32)
    assert x.shape == (B, C, H, W) and w.shape == (C, 1, KH, KW)

    # --- host-side exact prep (mirrors reference arithmetic in f32) ---
    x2 = (x / _mantissa_map(x)).astype(np.float32)  # x' : signed powers of two
    wr = w.reshape(C, KH, KW)
    wq = (wr / _mantissa_map(wr)).astype(np.float32)  # w'
    w2 = (wr - wq).astype(np.float32)  # w - w'

    # padded, K-stacked input layout: xin_all[u, c, b, col]
    xp = np.zeros((B, C, HP, HP), np.float32)
    xp[:, :, PAD : PAD + H, PAD : PAD + W] = x
    x2p = np.zeros((B, C, HP, HP), np.float32)
    x2p[:, :, PAD : PAD + H, PAD : PAD + W] = x2
    xin_all = np.empty((2 * HP, C, B, HP), dtype=BF16)
    xin_all[:HP] = xp.transpose(2, 1, 0, 3).astype(BF16)
    xin_all[HP:] = x2p.transpose(2, 1, 0, 3).astype(BF16)

    # banded Toeplitz expansion: tw_all[u, c, j, y]
    tw_all = np.zeros((2 * HP, C, KW, H), np.float32)
    yidx = np.arange(H)
    for i in range(KH):
        # advanced indexing on dims 0,3 -> result dims (y, c, j)
        tw_all[yidx + i, :, :, yidx] = np.broadcast_to(wq[:, i, :], (H, C, KW))
        tw_all[HP + yidx + i, :, :, yidx] = np.broadcast_to(w2[:, i, :], (H, C, KW))
    tw_all = tw_all.astype(BF16)

    nc = _build_graph()
    in_maps = [
        {
            "xin": np.ascontiguousarray(xin_all[:, k * CP : (k + 1) * CP]),
            "tw": np.ascontiguousarray(tw_all[:, k * CP : (k + 1) * CP]),
        }
        for k in range(N_CORES)
    ]
    res = run_bass_kernel_spmd(
        nc,
        in_maps,
        core_ids=list(range(N_CORES)),
        trace=bool(int(os.environ.get("KERNEL_TRACE", "0") or "0")),
    )
    LAST_RESULT = res

    y = np.stack([res.results[k]["out"] for k in range(N_CORES)])  # (8,H,CP,B,W)
    y = y.transpose(3, 0, 2, 1, 4).reshape(B, C, H, W)
    return np.ascontiguousarray(y.astype(np.float32))


# revision 11
# speedup vs baseline: 2.0966x; 1.4515x over previous
"""Approximate depthwise WAConv2d on 8 Trainium2 NeuronCores.

Math: the reference computes, per 7x7 depthwise patch,
    y = sum_k factor(a,b) * x*w,  factor = (a+b-1)/(a*b),
    a = mantissa_map(x), b = mantissa_map(w).
Since factor*x*w = x*(w/b) + (x/a)*w - (x/a)*(w/b), the whole op is exactly
    y = DWConv(x, w') + DWConv(x', w - w'),   x' = x/a, w' = w/b
i.e. two exact depthwise convolutions, fused into a single TensorEngine
matmul accumulation by stacking along the contraction dim.

Device formulation (per channel c, kernel-column j):
    out[y, (b,x)] += sum_r T_j[r, y] * Xcp_b[r, x+j]
where r runs over the 56 real input rows for x (partitions 0..55) and for
x' (partitions 56..111), Xcp is column-padded (62 cols), and T_j is the
banded Toeplitz expansion T_j[y+i-3, y] = w'[c,i,j] (and the w-w' half at
partition offset 56). 7 j-shifted matmuls (K=112, M=56, N=4*56=224)
accumulate in PSUM. Channels are sharded 24/core; data+weights stream in
graduated channel chunks over the gpsimd SWDGE ring so the PE never
starves; dummy warm-up matmuls keep the PE HAM clock at 2.4 GHz before
the first real chunk lands.
"""

import os
import sys

import numpy as np

if "/opt/trn_rl_repo" not in sys.path:
    sys.path.insert(0, "/opt/trn_rl_repo")

import ml_dtypes

B, C, H, W = 4, 192, 56, 56
KH = KW = 7
PAD = 3
WP = W + 2 * PAD  # 62 padded cols
N_CORES = 8
CP = C // N_CORES  # 24 channels per core
EPS = 1e-7

BF16 = ml_dtypes.bfloat16

CHUNKS = [2, 4, 6, 6, 4, 2]  # graduated channel chunks
assert sum(CHUNKS) == CP
N_WARM_MM = 20  # dummy matmuls to spin up the PE clock

_CACHE = {}
LAST_RESULT = None


def _mantissa_map(v):
    a = np.abs(v, dtype=np.float32) + np.float32(EPS)
    base = a * np.exp2(-np.floor(np.log2(a))) - np.float32(1.0)
    return np.where(base >= 0.5, (1.0 + base) * 0.5, 1.0 + base).astype(np.float32)


def _build_graph():
    """Build + compile the Bass graph once per process."""
    if "nc" in _CACHE:
        return _CACHE["nc"]

    import concourse.tile as tile
    from concourse import bacc, mybir

    nc = bacc.Bacc(
        "TRN2",
        target_bir_lowering=False,
        debug=False,
        enable_asserts=False,
        num_devices=N_CORES,
    )
    # channel-minor layouts so chunk DMAs have multi-KB contiguous lines
    xin = nc.dram_tensor(
        "xin", [2 * H, CP, B, WP], mybir.dt.bfloat16, kind="ExternalInput"
    ).ap()
    tw = nc.dram_tensor(
        "tw", [2 * H, CP, KW, H], mybir.dt.bfloat16, kind="ExternalInput"
    ).ap()
    out = nc.dram_tensor(
        "out", [H, CP, B, W], mybir.dt.float32, kind="ExternalOutput"
    ).ap()

    with tile.TileContext(nc) as tc:
        with (
            tc.tile_pool(name="xp", bufs=len(CHUNKS)) as xpool,
            tc.tile_pool(name="tp", bufs=len(CHUNKS)) as tpool,
            tc.tile_pool(name="pp", bufs=7, space="PSUM") as ppool,
            tc.tile_pool(name="op", bufs=len(CHUNKS)) as opool,
            tc.tile_pool(name="wp", bufs=1) as wpool,
        ):
            # PE warm-up: dummy matmuls on an (uninitialized) scratch tile.
            # No data deps -> they dispatch right after the preamble, so the
            # HAM activity window up-clocks the PE before real work arrives.
            warm = wpool.tile([2 * H, 224], mybir.dt.bfloat16)
            nc.vector.memset(warm[:], 0.0)
            wps = ppool.tile([H, 224], mybir.dt.float32, tag="warm", bufs=1)
            for _ in range(N_WARM_MM):
                nc.tensor.matmul(
                    wps[:], warm[:, :H], warm[:, :224], start=True, stop=True
                )

            xts, tts = [], []
            c0 = 0
            # all input DMAs on the gpsimd SWDGE ring, interleaved x/t in
            # chunk order -> FIFO completion, early chunks land first
            for sz in CHUNKS:
                cs = slice(c0, c0 + sz)
                xt = xpool.tile([2 * H, sz, B, WP], mybir.dt.bfloat16, tag=f"x{sz}")
                nc.gpsimd.dma_start(out=xt[:], in_=xin[:, cs])
                tt = tpool.tile([2 * H, sz, KW, H], mybir.dt.bfloat16, tag=f"t{sz}")
                nc.gpsimd.dma_start(out=tt[:], in_=tw[:, cs])
                xts.append(xt)
                tts.append(tt)
                c0 += sz
            c0 = 0
            for g, sz in enumerate(CHUNKS):
                cs = slice(c0, c0 + sz)
                xt, tt = xts[g], tts[g]
                ot = opool.tile([H, sz, B, W], mybir.dt.float32, tag=f"o{sz}")
                for ci in range(sz):
                    ps = ppool.tile([H, B, W], mybir.dt.float32)
                    for j in range(KW):
                        nc.tensor.matmul(
                            ps[:],
                            tt[:, ci, j, :],
                            xt[:, ci, :, j : j + W],
                            start=(j == 0),
                            stop=(j == KW - 1),
                        )
                    nc.vector.tensor_copy(ot[:, ci], ps[:])
                nc.scalar.dma_start(out=out[:, cs], in_=ot[:])
                c0 += sz

    nc.compile()
    _CACHE["nc"] = nc
    return nc


def kernel(x, weight):
    global LAST_RESULT
    from concourse.bass_utils import run_bass_kernel_spmd

    x = np.asarray(x, dtype=np.float32)
    w = np.asarray(weight, dtype=np.float32)
    assert x.shape == (B, C, H, W) and w.shape == (C, 1, KH, KW)

    # --- host-side exact prep (mirrors reference arithmetic in f32) ---
    x2 = (x / _mantissa_map(x)).astype(np.float32)  # x' : signed powers of two
    wr = w.reshape(C, KH, KW)
    wq = (wr / _mantissa_map(wr)).astype(np.float32)  # w'
    w2 = (wr - wq).astype(np.float32)  # w - w'

    # column-padded, K-stacked input layout: xin_all[r, c, b, col]
    xp = np.zeros((B, C, H, WP), np.float32)
    xp[:, :, :, PAD : PAD + W] = x
    x2p = np.zeros((B, C, H, WP), np.float32)
    x2p[:, :, :, PAD : PAD + W] = x2
    xin_all = np.empty((2 * H, C, B, WP), dtype=BF16)
    xin_all[:H] = xp.transpose(2, 1, 0, 3).astype(BF16)
    xin_all[H:] = x2p.transpose(2, 1, 0, 3).astype(BF16)

    # banded Toeplitz expansion: tw_all[r, c, j, y] = w~[c, r+3-y, j]
    tw_all = np.zeros((2 * H, C, KW, H), np.float32)
    for i in range(KH):
        # rows r = y + i - 3 for valid y
        ys = np.arange(max(0, 3 - i), min(H, H + 3 - i))
        rs = ys + i - 3
        tw_all[rs, :, :, ys] = np.broadcast_to(wq[:, i, :], (len(ys), C, KW))
        tw_all[H + rs, :, :, ys] = np.broadcast_to(w2[:, i, :], (len(ys), C, KW))
    tw_all = tw_all.astype(BF16)

    nc = _build_graph()
    in_maps = [
        {
            "xin": np.ascontiguousarray(xin_all[:, k * CP : (k + 1) * CP]),
            "tw": np.ascontiguousarray(tw_all[:, k * CP : (k + 1) * CP]),
        }
        for k in range(N_CORES)
    ]
    res = run_bass_kernel_spmd(
        nc,
        in_maps,
        core_ids=list(range(N_CORES)),
        trace=bool(int(os.environ.get("KERNEL_TRACE", "0") or "0")),
    )
    LAST_RESULT = res

    y = np.stack([res.results[k]["out"] for k in range(N_CORES)])  # (8,H,CP,B,W)
    y = y.transpose(3, 0, 2, 1, 4).reshape(B, C, H, W)
    return np.ascontiguousarray(y.astype(np.float32))
